# revision 33
# baseline (speedup 1.0000x reference)
"""Trainium2 Bass kernel for CAM-style channel attention module.

Reference computation (per batch b):
    Q  = W @ X + bias          # 1x1 conv: [256,512]@[512,4096] -> [256,4096]
    E  = Q @ X^T / sqrt(4096)  # [256,512] channel-attention energy
    A  = softmax(E, axis=-1)
    out = gamma * (A @ X) + Q  # residual

Two algebraic folds carry the whole design:
  1. gamma*(A@X) + (W@X + b) = (W + gamma*A) @ X + b  -- the residual Q is
     never materialized; the final stage is one fused bf16 matmul.
  2. E = (W@X + b) @ X^T = W*(X@X^T) + b*rowsum(X)^T = W*G + b*s^T.
     G = X@X^T is SYMMETRIC, so only its upper triangle is computed
     (wide rows of width 512/384/256/128 -> 40960 PE cycles vs 65536 for
     the Q^T + energy pipeline), the lower triangle comes from 6 small
     PE transposes, and W*G runs in fp16 (10-bit mantissa beats bf16's 8,
     full bf16-rate streaming; fp32r was rejected by the BIR verifier's
     producer-rounding rule).  b*s^T is a rank-1 update: two K=1 bf16
     matmuls with host-precomputed s.

X is uploaded host-pre-cast bf16 in natural [c,n] (final-matmul rhs)
and pre-transposed [n,c] (Gram operands) layouts; output written bf16.
Measured numerics (numpy emulation): rel ~ 0.006 vs the 0.02 gate --
considerably better than the all-bf16 QT+energy pipeline (0.013).

Per core: PE ~ 82.9k cyc/batch = 69us busy, DMA 21 MB = 59us.
"""

import numpy as np
import ml_dtypes

import concourse.bass as bass
import concourse.tile as tile
from concourse import bacc, mybir
from concourse.bass_utils import run_bass_kernel_spmd

P = 128
NB = 2         # batches per core (B=16 over 8 cores)
C = 512        # input channels
C1 = 256       # conv output channels
HW = 4096      # H*W
CT = C // P    # 4 c-tiles
NT = HW // P   # 32 n-tiles
QT = C1 // P   # 2 q-tiles
XCH = 512      # column chunk (one PSUM bank / nice DMA size)
NCHK = HW // XCH  # 8 chunks
F32 = mybir.dt.float32
F16 = mybir.dt.float16
BF16 = mybir.dt.bfloat16
SCALE = 1.0 / 64.0   # 1/sqrt(HW)

N_CORES = 8


def build_nc():
    nc = bacc.Bacc("TRN2", target_bir_lowering=False, debug=False,
                   num_devices=N_CORES)

    xbfd = nc.dram_tensor("xbfd", [NB, P, CT, HW], BF16,
                          kind="ExternalInput").ap()
    xtbd = nc.dram_tensor("xtbd", [NB, P, NT, C], BF16,
                          kind="ExternalInput").ap()
    wtfd = nc.dram_tensor("wtfd", [P, CT, C1], F32, kind="ExternalInput").ap()
    wthd = nc.dram_tensor("wthd", [P, CT, C1], F16, kind="ExternalInput").ap()
    bqd = nc.dram_tensor("bqd", [P, QT], F32, kind="ExternalInput").ap()
    gamd = nc.dram_tensor("gamd", [P, 1], F32, kind="ExternalInput").ap()
    bbd = nc.dram_tensor("bbd", [1, C1], BF16, kind="ExternalInput").ap()
    ssd = nc.dram_tensor("ssd", [1, NB, C], BF16, kind="ExternalInput").ap()
    out = nc.dram_tensor("out", [NB, C1, HW], BF16, kind="ExternalOutput").ap()

    ident_dram = nc.inline_tensor(np.eye(P, dtype=ml_dtypes.bfloat16),
                                  name="ident")
    identh_dram = nc.inline_tensor(np.eye(P, dtype=np.float16),
                                   name="identh")

    with tile.TileContext(nc) as tc:
        with (
            tc.tile_pool(name="const", bufs=1) as const,
            tc.tile_pool(name="xbfp", bufs=2 * NCHK) as xbfp,
            tc.tile_pool(name="xtbp", bufs=2 * NCHK) as xtbp,
            tc.tile_pool(name="gsb", bufs=2) as gsb_pool,
            tc.tile_pool(name="smp", bufs=2) as smp,
            tc.tile_pool(name="lhsfp", bufs=2) as lhsfp,
            tc.tile_pool(name="osbp", bufs=4) as osbp,
            tc.tile_pool(name="psG", bufs=4, space="PSUM") as psG,
            tc.tile_pool(name="psE", bufs=2, space="PSUM") as psE,
            tc.tile_pool(name="psO", bufs=2, space="PSUM") as psO,
        ):
            # ---- constants: none are needed before ~15us (WG/AT/F), so
            # they all queue AFTER batch-0's xtb chunks -> chunk0 lands
            # ~2us earlier and the Gram stage starts sooner.
            wtf_sb = const.tile([P, CT, C1], F32)
            wth_sb = const.tile([P, CT, C1], F16)
            ident = const.tile([P, P], BF16)
            identh = const.tile([P, P], F16)
            bq_sb = const.tile([P, QT], F32)
            gam_sb = const.tile([P, 1], F32)
            bb_sb = const.tile([1, C1], BF16)
            ss_sb = const.tile([1, NB, C], BF16, name="ss_sb")

            def issue_late_consts():
                nc.sync.dma_start(out=wth_sb, in_=wthd)
                nc.sync.dma_start(out=identh, in_=identh_dram.ap())
                nc.sync.dma_start(out=ident, in_=ident_dram.ap())
                nc.sync.dma_start(out=bb_sb, in_=bbd)
                nc.sync.dma_start(out=ss_sb, in_=ssd)
                nc.sync.dma_start(out=gam_sb, in_=gamd)
                nc.sync.dma_start(out=wtf_sb, in_=wtfd)
                nc.sync.dma_start(out=bq_sb, in_=bqd)

            out_r = out.rearrange("b (t p) n -> b p t n", p=P)

            st = [dict() for _ in range(NB)]

            def issue_xtb_dma(bi):
                xtbc = []
                for j in range(NCHK):
                    b = xtbp.tile([P, 4, C], BF16, tag="xtb",
                                  name=f"xtb_{bi}_{j}")
                    nc.sync.dma_start(
                        out=b, in_=xtbd[bi][:, 4 * j:4 * (j + 1), :])
                    xtbc.append(b)
                st[bi]["xtb"] = xtbc

            def issue_xbf_dma(bi):
                xbfc = []
                for j in range(NCHK):
                    a = xbfp.tile([P, CT, XCH], BF16, tag="xbf",
                                  name=f"xbf_{bi}_{j}")
                    nc.sync.dma_start(
                        out=a, in_=xbfd[bi][:, :, j * XCH:(j + 1) * XCH])
                    xbfc.append(a)
                st[bi]["xbf"] = xbfc

            # ---- Gram upper-triangle rows, accumulated chunk by chunk ----
            def emit_G_start(bi):
                st[bi]["ps_g"] = [
                    psG.tile([P, C], F32, tag="g", name=f"ps_g{bi}{r}")
                    for r in range(CT)]

            def emit_G_chunk(bi, j):
                """Accumulate xtb chunk j (4 n-tiles) into all 4 G rows."""
                ch = st[bi]["xtb"][j]
                for nt in range(4):
                    for r in range(CT):
                        nc.tensor.matmul(
                            st[bi]["ps_g"][r][:, :C - r * P],
                            ch[:, nt, r * P:(r + 1) * P],
                            ch[:, nt, r * P:],
                            start=(j == 0 and nt == 0),
                            stop=(j == NCHK - 1 and nt == 3))

            def emit_G_evac(bi):
                """Evacuate the 4 upper-triangle rows (scalar/DVE).
                Row 0 gates the next batch's Gram start (its psum bank is
                reused first), so its evac is split across both engines."""
                g_sb = gsb_pool.tile([P, CT, C], F16, tag="g",
                                     name=f"g_sb{bi}")
                src0 = st[bi]["ps_g"][0]
                nc.scalar.copy(out=g_sb[:, 0, :C // 2], in_=src0[:, :C // 2])
                nc.vector.tensor_copy(g_sb[:, 0, C // 2:], src0[:, C // 2:])
                for r in range(1, CT):
                    src = st[bi]["ps_g"][r][:, :C - r * P]
                    if r % 2 == 0:
                        nc.scalar.copy(out=g_sb[:, r, r * P:], in_=src)
                    else:
                        nc.vector.tensor_copy(g_sb[:, r, r * P:], src)
                st[bi]["g_sb"] = g_sb

            def emit_G_lower(bi):
                """Transpose-fill the lower triangle from the upper rows."""
                g_sb = st[bi]["g_sb"]
                for ci in range(1, CT):
                    for cj in range(ci):
                        # transpose psums live in the e-banks (free here),
                        # keeping the po rotation clear for AT/F
                        ps_t = psE.tile([P, C], F32, tag="e", name="ps_t")
                        nc.tensor.matmul(
                            ps_t[:, :P],
                            g_sb[:, cj, ci * P:(ci + 1) * P],
                            identh, start=True, stop=True)
                        nc.vector.tensor_copy(
                            g_sb[:, ci, cj * P:(cj + 1) * P], ps_t[:, :P])

            # ---- E = W*G (fp16) + b*s^T (rank-1 bf16) ----
            def emit_WG(bi):
                g_r = st[bi]["g_sb"]
                w_r = wth_sb
                st[bi]["ps_e"] = []
                for qi in range(QT):
                    ps_e = psE.tile([P, C], F32, tag="e",
                                    name=f"ps_e{bi}{qi}")
                    for ct in range(CT):
                        nc.tensor.matmul(
                            ps_e, w_r[:, ct, qi * P:(qi + 1) * P],
                            g_r[:, ct, :],
                            start=(ct == 0), stop=False)
                    nc.tensor.matmul(
                        ps_e, bb_sb[:, qi * P:(qi + 1) * P],
                        ss_sb[:, bi, :], start=False, stop=True)
                    st[bi]["ps_e"].append(ps_e)

            # ---- softmax + gamma/rowsum fold ----
            def emit_softmax(bi):
                a_scaled = smp.tile([P, QT, C], BF16, tag="a",
                                    name=f"a_scaled{bi}")
                for qi in range(QT):
                    ps_e = st[bi]["ps_e"][qi]
                    mx = smp.tile([P, 1], F32, tag="mx", name="mx")
                    nc.vector.reduce_max(mx, ps_e,
                                         axis=mybir.AxisListType.X,
                                         negate=True)
                    nbias = smp.tile([P, 1], F32, tag="nb", name="nb")
                    nc.vector.tensor_scalar_mul(nbias, mx, SCALE)
                    a_f = smp.tile([P, C], F32, tag="af", name="a_f")
                    rs = smp.tile([P, 1], F32, tag="rs", name="rs")
                    nc.scalar.activation(
                        out=a_f, in_=ps_e,
                        func=mybir.ActivationFunctionType.Exp,
                        bias=nbias, scale=SCALE, accum_out=rs)
                    rc = smp.tile([P, 1], F32, tag="rc", name="rc")
                    nc.vector.reciprocal(rc, rs)
                    sc = smp.tile([P, 1], F32, tag="sc", name="sc")
                    nc.vector.tensor_mul(sc, rc, gam_sb)
                    nc.vector.tensor_scalar_mul(a_scaled[:, qi, :], a_f, sc)
                st[bi]["a"] = a_scaled

            # ---- A^T via regular matmul vs identity; combine with W^T ----
            def emit_ATcombine(bi):
                lhsf = lhsfp.tile([P, CT, C1], BF16, tag="lhsf",
                                  name=f"lhsf{bi}")
                a_scaled = st[bi]["a"]
                for ct in range(CT):
                    ps_at = psO.tile([P, XCH], F32, tag="po", name="ps_at")
                    for qi in range(QT):
                        nc.tensor.matmul(
                            ps_at[:, qi * P:(qi + 1) * P],
                            a_scaled[:, qi, ct * P:(ct + 1) * P], ident,
                            start=(qi == 0), stop=(qi == QT - 1))
                    nc.vector.tensor_add(
                        out=lhsf[:, ct, :], in0=ps_at[:, :C1],
                        in1=wtf_sb[:, ct, :])
                st[bi]["lhsf"] = lhsf

            # ---- final fused matmul: (W + gamma*A) @ X + b, bf16 ----
            def emit_F(bi, qi, half, deep=False, fine=False):
                # deep=True (batch 1 tail): alternate psum between the po
                # and the then-free e banks for a 4-deep rotation.
                # fine=True (very last group): 256-col sub-chunks so the
                # final evac+store tail is half as long.
                lhsf = st[bi]["lhsf"]
                o_sb = osbp.tile([P, 4 * XCH], BF16, tag="o", name="o_sb")
                w = XCH // 2 if fine else XCH
                nsub = (4 * XCH) // w
                for j in range(nsub):
                    lo = half * 4 * XCH + j * w
                    if deep and j % 2 == 1:
                        ps_o = psE.tile([P, C], F32, tag="e", name="ps_o_e")
                    else:
                        ps_o = psO.tile([P, XCH], F32, tag="po", name="ps_o")
                    rhs = st[bi]["xbf"][lo // XCH]
                    roff = lo % XCH
                    for ct in range(CT):
                        nc.tensor.matmul(
                            ps_o[:, :w], lhsf[:, ct, qi * P:(qi + 1) * P],
                            rhs[:, ct, roff:roff + w],
                            start=(ct == 0), stop=(ct == CT - 1))
                    oslice = o_sb[:, j * w:(j + 1) * w]
                    if j % 2 == 0:
                        nc.scalar.add(out=oslice, in_=ps_o[:, :w],
                                      add=bq_sb[:, qi:qi + 1])
                    else:
                        nc.vector.tensor_scalar_add(oslice, ps_o[:, :w],
                                                    bq_sb[:, qi:qi + 1])
                    nc.scalar.dma_start(
                        out=out_r[bi, :, qi, lo:lo + w],
                        in_=oslice)

            # ---- HAM warm-up on a memset tile (no DMA dependency).
            # ~6k cycles of dummy matmuls: the hw clock governor needs
            # sustained PE work before it ramps to 2.4 GHz (measured:
            # a 20-instruction warmup cost +8us of slow-clock matmuls).
            warm_sb = const.tile([P, P], BF16, name="warm_sb")
            nc.vector.memset(warm_sb, 0.0)
            ps_w = psO.tile([P, XCH], F32, tag="po", name="warm")
            NWARM = 44
            for wj in range(NWARM):
                nc.tensor.matmul(ps_w[:, :P], warm_sb, warm_sb,
                                 start=(wj == 0), stop=(wj == NWARM - 1))
            # warm the Exp activation table while the PE warms up
            dummy_e = smp.tile([P, 1], F32, tag="rs", name="dummy_e")
            nc.scalar.activation(out=dummy_e, in_=warm_sb[:, 0:1],
                                 func=mybir.ActivationFunctionType.Exp)

            # ---- DMA spine ----
            issue_xtb_dma(0)
            issue_late_consts()
            issue_xtb_dma(1)
            issue_xbf_dma(0)
            issue_xbf_dma(1)

            # ---- PE schedule ----
            emit_G_start(0)
            for j in range(NCHK):
                emit_G_chunk(0, j)
            emit_G_evac(0)
            emit_G_start(1)
            emit_G_chunk(1, 0)    # PE covers batch-0's row-evac latency
            emit_G_lower(0)
            emit_WG(0)
            emit_softmax(0)
            emit_G_chunk(1, 1)    # fill softmax(0) latency
            emit_G_chunk(1, 2)
            emit_ATcombine(0)
            emit_F(0, 0, 0)
            emit_G_chunk(1, 3)
            emit_G_chunk(1, 4)
            emit_F(0, 0, 1)
            emit_G_chunk(1, 5)
            emit_G_chunk(1, 6)
            emit_G_chunk(1, 7)
            emit_G_evac(1)
            emit_F(0, 1, 0)       # PE covers batch-1's row-evac latency
            emit_G_lower(1)
            emit_WG(1)
            emit_softmax(1)
            emit_F(0, 1, 1)       # covers softmax(1) latency
            emit_ATcombine(1)
            emit_F(1, 0, 0, deep=True)
            emit_F(1, 0, 1, deep=True)
            emit_F(1, 1, 0, deep=True)
            emit_F(1, 1, 1, deep=True, fine=True)
    nc.compile()
    return nc


_NC_CACHE = None


def _get_nc():
    global _NC_CACHE
    if _NC_CACHE is None:
        _NC_CACHE = build_nc()
    return _NC_CACHE


def make_in_maps(x, conv_w, conv_b, gamma):
    B = x.shape[0]
    xs = np.ascontiguousarray(x.reshape(B, C, HW), dtype=np.float32)
    # natural layout, p = c % 128 partition: [B, P, CT, HW]
    xn = xs.reshape(B, CT, P, HW).transpose(0, 2, 1, 3)
    xbf = np.ascontiguousarray(xn).astype(ml_dtypes.bfloat16)
    # transposed layout, p = n % 128 partition: [B, P, NT, C]
    xt = xs.transpose(0, 2, 1).reshape(B, NT, P, C).transpose(0, 2, 1, 3)
    xtb = np.ascontiguousarray(xt).astype(ml_dtypes.bfloat16)
    # per-batch channel row-sums for the rank-1 bias term of E
    ss = xs.sum(axis=2).astype(ml_dtypes.bfloat16)     # [B, C]

    wm = conv_w.reshape(C1, C).astype(np.float32)
    wt = np.ascontiguousarray(wm.T)                    # [C, C1]
    wt_tiled = np.ascontiguousarray(
        wt.reshape(CT, P, C1).transpose(1, 0, 2))      # [P, CT, C1]
    b_np = conv_b.astype(np.float32)
    bq = np.ascontiguousarray(b_np.reshape(QT, P).T)   # [P, QT]
    bb = np.ascontiguousarray(b_np[None, :]).astype(ml_dtypes.bfloat16)
    gam = np.ascontiguousarray(
        np.broadcast_to(gamma.astype(np.float32).reshape(1, 1), (P, 1)))
    in_maps = []
    for ci in range(N_CORES):
        sl = slice(NB * ci, NB * (ci + 1))
        in_maps.append({
            "xbfd": np.ascontiguousarray(xbf[sl]),
            "xtbd": np.ascontiguousarray(xtb[sl]),
            "wtfd": wt_tiled,
            "wthd": wt_tiled.astype(np.float16),
            "bqd": bq,
            "gamd": gam,
            "bbd": bb,
            "ssd": np.ascontiguousarray(ss[sl]).reshape(1, NB, C),
        })
    return in_maps


def kernel(x, conv_w, conv_b, gamma, trace=False):
    """Full inputs in, full output out. Shards batch over 8 NeuronCores."""
    nc = _get_nc()
    in_maps = make_in_maps(x, conv_w, conv_b, gamma)
    res = run_bass_kernel_spmd(nc, in_maps, core_ids=list(range(N_CORES)),
                               trace=trace)
    outs = [np.asarray(r["out"]).astype(np.float32).reshape(NB, C1, 64, 64)
            for r in res.results]
    full = np.concatenate(outs, axis=0)
    if trace:
        kernel.last_results = res
    return full


kernel.last_results = None


# revision 34
# speedup vs baseline: 1.0154x; 1.0154x over previous
"""Trainium2 Bass kernel for CAM-style channel attention module.

Reference computation (per batch b):
    Q  = W @ X + bias          # 1x1 conv: [256,512]@[512,4096] -> [256,4096]
    E  = Q @ X^T / sqrt(4096)  # [256,512] channel-attention energy
    A  = softmax(E, axis=-1)
    out = gamma * (A @ X) + Q  # residual

Two algebraic folds carry the whole design:
  1. gamma*(A@X) + (W@X + b) = (W + gamma*A) @ X + b  -- the residual Q is
     never materialized; the final stage is one fused bf16 matmul.
  2. E = (W@X + b) @ X^T = W*(X@X^T) + b*rowsum(X)^T = W*G + b*s^T.
     G = X@X^T is SYMMETRIC, so only its upper triangle is computed
     (wide rows of width 512/384/256/128 -> 40960 PE cycles vs 65536 for
     the Q^T + energy pipeline), the lower triangle comes from 6 small
     PE transposes, and W*G runs in fp16 (10-bit mantissa beats bf16's 8,
     full bf16-rate streaming; fp32r was rejected by the BIR verifier's
     producer-rounding rule).  b*s^T is a rank-1 update: two K=1 bf16
     matmuls with host-precomputed s.

X is uploaded host-pre-cast bf16 in natural [c,n] (final-matmul rhs)
and pre-transposed [n,c] (Gram operands) layouts; output written bf16.
Measured numerics (numpy emulation): rel ~ 0.006 vs the 0.02 gate --
considerably better than the all-bf16 QT+energy pipeline (0.013).

Per core: PE ~ 82.9k cyc/batch = 69us busy, DMA 21 MB = 59us.
"""

import numpy as np
import ml_dtypes

import concourse.bass as bass
import concourse.tile as tile
from concourse import bacc, mybir
from concourse.bass_utils import run_bass_kernel_spmd

P = 128
NB = 2         # batches per core (B=16 over 8 cores)
C = 512        # input channels
C1 = 256       # conv output channels
HW = 4096      # H*W
CT = C // P    # 4 c-tiles
NT = HW // P   # 32 n-tiles
QT = C1 // P   # 2 q-tiles
XCH = 512      # column chunk (one PSUM bank / nice DMA size)
NCHK = HW // XCH  # 8 chunks
F32 = mybir.dt.float32
F16 = mybir.dt.float16
BF16 = mybir.dt.bfloat16
SCALE = 1.0 / 64.0   # 1/sqrt(HW)

N_CORES = 8


def build_nc():
    nc = bacc.Bacc("TRN2", target_bir_lowering=False, debug=False,
                   num_devices=N_CORES)

    xbfd = nc.dram_tensor("xbfd", [NB, P, CT, HW], BF16,
                          kind="ExternalInput").ap()
    xtbd = nc.dram_tensor("xtbd", [NB, P, NT, C], BF16,
                          kind="ExternalInput").ap()
    wtfd = nc.dram_tensor("wtfd", [P, CT, C1], F32, kind="ExternalInput").ap()
    wthd = nc.dram_tensor("wthd", [P, CT, C1], F16, kind="ExternalInput").ap()
    bqd = nc.dram_tensor("bqd", [P, QT], F32, kind="ExternalInput").ap()
    gamd = nc.dram_tensor("gamd", [P, 1], F32, kind="ExternalInput").ap()
    bbd = nc.dram_tensor("bbd", [1, C1], BF16, kind="ExternalInput").ap()
    ssd = nc.dram_tensor("ssd", [1, NB, C], BF16, kind="ExternalInput").ap()
    out = nc.dram_tensor("out", [NB, C1, HW], BF16, kind="ExternalOutput").ap()

    ident_dram = nc.inline_tensor(np.eye(P, dtype=ml_dtypes.bfloat16),
                                  name="ident")
    identh_dram = nc.inline_tensor(np.eye(P, dtype=np.float16),
                                   name="identh")

    with tile.TileContext(nc) as tc:
        with (
            tc.tile_pool(name="const", bufs=1) as const,
            tc.tile_pool(name="xbfp", bufs=2 * NCHK) as xbfp,
            tc.tile_pool(name="xtbp", bufs=2 * NCHK) as xtbp,
            tc.tile_pool(name="gsb", bufs=2) as gsb_pool,
            tc.tile_pool(name="smp", bufs=2) as smp,
            tc.tile_pool(name="lhsfp", bufs=2) as lhsfp,
            tc.tile_pool(name="osbp", bufs=4) as osbp,
            tc.tile_pool(name="psG", bufs=4, space="PSUM") as psG,
            tc.tile_pool(name="psE", bufs=2, space="PSUM") as psE,
            tc.tile_pool(name="psO", bufs=2, space="PSUM") as psO,
        ):
            # ---- constants: none are needed before ~15us (WG/AT/F), so
            # they all queue AFTER batch-0's xtb chunks -> chunk0 lands
            # ~2us earlier and the Gram stage starts sooner.
            wtf_sb = const.tile([P, CT, C1], F32)
            wth_sb = const.tile([P, CT, C1], F16)
            ident = const.tile([P, P], BF16)
            identh = const.tile([P, P], F16)
            bq_sb = const.tile([P, QT], F32)
            gam_sb = const.tile([P, 1], F32)
            bb_sb = const.tile([1, C1], BF16)
            ss_sb = const.tile([1, NB, C], BF16, name="ss_sb")

            def issue_late_consts():
                nc.sync.dma_start(out=wth_sb, in_=wthd)
                nc.sync.dma_start(out=identh, in_=identh_dram.ap())
                nc.sync.dma_start(out=ident, in_=ident_dram.ap())
                nc.sync.dma_start(out=bb_sb, in_=bbd)
                nc.sync.dma_start(out=ss_sb, in_=ssd)
                nc.sync.dma_start(out=gam_sb, in_=gamd)
                nc.sync.dma_start(out=wtf_sb, in_=wtfd)
                nc.sync.dma_start(out=bq_sb, in_=bqd)

            out_r = out.rearrange("b (t p) n -> b p t n", p=P)

            st = [dict() for _ in range(NB)]

            def issue_xtb_dma(bi):
                xtbc = []
                for j in range(NCHK):
                    b = xtbp.tile([P, 4, C], BF16, tag="xtb",
                                  name=f"xtb_{bi}_{j}")
                    nc.sync.dma_start(
                        out=b, in_=xtbd[bi][:, 4 * j:4 * (j + 1), :])
                    xtbc.append(b)
                st[bi]["xtb"] = xtbc

            def issue_xbf_dma(bi):
                xbfc = []
                for j in range(NCHK):
                    a = xbfp.tile([P, CT, XCH], BF16, tag="xbf",
                                  name=f"xbf_{bi}_{j}")
                    nc.sync.dma_start(
                        out=a, in_=xbfd[bi][:, :, j * XCH:(j + 1) * XCH])
                    xbfc.append(a)
                st[bi]["xbf"] = xbfc

            # ---- Gram upper-triangle rows, accumulated chunk by chunk ----
            def emit_G_start(bi):
                st[bi]["ps_g"] = [
                    psG.tile([P, C], F32, tag="g", name=f"ps_g{bi}{r}")
                    for r in range(CT)]

            def emit_G_chunk(bi, j):
                """Accumulate xtb chunk j (4 n-tiles) into all 4 G rows."""
                ch = st[bi]["xtb"][j]
                for nt in range(4):
                    for r in range(CT):
                        nc.tensor.matmul(
                            st[bi]["ps_g"][r][:, :C - r * P],
                            ch[:, nt, r * P:(r + 1) * P],
                            ch[:, nt, r * P:],
                            start=(j == 0 and nt == 0),
                            stop=(j == NCHK - 1 and nt == 3))

            def emit_G_evac(bi):
                """Evacuate the 4 upper-triangle rows (scalar/DVE).
                Row 0 gates the next batch's Gram start (its psum bank is
                reused first), so its evac is split across both engines."""
                g_sb = gsb_pool.tile([P, CT, C], F16, tag="g",
                                     name=f"g_sb{bi}")
                src0 = st[bi]["ps_g"][0]
                nc.scalar.copy(out=g_sb[:, 0, :C // 2], in_=src0[:, :C // 2])
                nc.vector.tensor_copy(g_sb[:, 0, C // 2:], src0[:, C // 2:])
                for r in range(1, CT):
                    src = st[bi]["ps_g"][r][:, :C - r * P]
                    if r % 2 == 0:
                        nc.scalar.copy(out=g_sb[:, r, r * P:], in_=src)
                    else:
                        nc.vector.tensor_copy(g_sb[:, r, r * P:], src)
                st[bi]["g_sb"] = g_sb

            def emit_G_lower(bi):
                """Transpose-fill the lower triangle from the upper rows."""
                g_sb = st[bi]["g_sb"]
                for ci in range(1, CT):
                    for cj in range(ci):
                        # transpose psums live in the e-banks (free here),
                        # keeping the po rotation clear for AT/F
                        ps_t = psE.tile([P, C], F32, tag="e", name="ps_t")
                        nc.tensor.matmul(
                            ps_t[:, :P],
                            g_sb[:, cj, ci * P:(ci + 1) * P],
                            identh, start=True, stop=True)
                        nc.vector.tensor_copy(
                            g_sb[:, ci, cj * P:(cj + 1) * P], ps_t[:, :P])

            # ---- E = W*G (fp16) + b*s^T (rank-1 bf16) ----
            def emit_WG(bi):
                g_r = st[bi]["g_sb"]
                w_r = wth_sb
                st[bi]["ps_e"] = []
                for qi in range(QT):
                    ps_e = psE.tile([P, C], F32, tag="e",
                                    name=f"ps_e{bi}{qi}")
                    for ct in range(CT):
                        nc.tensor.matmul(
                            ps_e, w_r[:, ct, qi * P:(qi + 1) * P],
                            g_r[:, ct, :],
                            start=(ct == 0), stop=False)
                    nc.tensor.matmul(
                        ps_e, bb_sb[:, qi * P:(qi + 1) * P],
                        ss_sb[:, bi, :], start=False, stop=True)
                    st[bi]["ps_e"].append(ps_e)

            # ---- softmax + gamma/rowsum fold ----
            def emit_softmax(bi):
                a_scaled = smp.tile([P, QT, C], BF16, tag="a",
                                    name=f"a_scaled{bi}")
                for qi in range(QT):
                    ps_e = st[bi]["ps_e"][qi]
                    mx = smp.tile([P, 1], F32, tag="mx", name="mx")
                    nc.vector.reduce_max(mx, ps_e,
                                         axis=mybir.AxisListType.X,
                                         negate=True)
                    nbias = smp.tile([P, 1], F32, tag="nb", name="nb")
                    nc.vector.tensor_scalar_mul(nbias, mx, SCALE)
                    a_f = smp.tile([P, C], F32, tag="af", name="a_f")
                    rs = smp.tile([P, 1], F32, tag="rs", name="rs")
                    nc.scalar.activation(
                        out=a_f, in_=ps_e,
                        func=mybir.ActivationFunctionType.Exp,
                        bias=nbias, scale=SCALE, accum_out=rs)
                    rc = smp.tile([P, 1], F32, tag="rc", name="rc")
                    nc.vector.reciprocal(rc, rs)
                    sc = smp.tile([P, 1], F32, tag="sc", name="sc")
                    nc.vector.tensor_mul(sc, rc, gam_sb)
                    nc.vector.tensor_scalar_mul(a_scaled[:, qi, :], a_f, sc)
                st[bi]["a"] = a_scaled

            # ---- A^T via regular matmul vs identity; combine with W^T ----
            def emit_ATcombine(bi):
                lhsf = lhsfp.tile([P, CT, C1], BF16, tag="lhsf",
                                  name=f"lhsf{bi}")
                a_scaled = st[bi]["a"]
                for ct in range(CT):
                    ps_at = psO.tile([P, XCH], F32, tag="po", name="ps_at")
                    for qi in range(QT):
                        nc.tensor.matmul(
                            ps_at[:, qi * P:(qi + 1) * P],
                            a_scaled[:, qi, ct * P:(ct + 1) * P], ident,
                            start=(qi == 0), stop=(qi == QT - 1))
                    nc.vector.tensor_add(
                        out=lhsf[:, ct, :], in0=ps_at[:, :C1],
                        in1=wtf_sb[:, ct, :])
                st[bi]["lhsf"] = lhsf

            # ---- final fused matmul: (W + gamma*A) @ X + b, bf16 ----
            def emit_F(bi, qi, half, deep=False, fine=False):
                # deep=True (batch 1 tail): alternate psum between the po
                # and the then-free e banks for a 4-deep rotation.
                # fine=True (very last group): 256-col sub-chunks so the
                # final evac+store tail is half as long.
                lhsf = st[bi]["lhsf"]
                o_sb = osbp.tile([P, 4 * XCH], BF16, tag="o", name="o_sb")
                w = XCH // 2 if fine else XCH
                nsub = (4 * XCH) // w
                for j in range(nsub):
                    lo = half * 4 * XCH + j * w
                    if deep and j % 2 == 1:
                        ps_o = psE.tile([P, C], F32, tag="e", name="ps_o_e")
                    else:
                        ps_o = psO.tile([P, XCH], F32, tag="po", name="ps_o")
                    rhs = st[bi]["xbf"][lo // XCH]
                    roff = lo % XCH
                    for ct in range(CT):
                        nc.tensor.matmul(
                            ps_o[:, :w], lhsf[:, ct, qi * P:(qi + 1) * P],
                            rhs[:, ct, roff:roff + w],
                            start=(ct == 0), stop=(ct == CT - 1))
                    oslice = o_sb[:, j * w:(j + 1) * w]
                    if j % 2 == 0:
                        nc.scalar.add(out=oslice, in_=ps_o[:, :w],
                                      add=bq_sb[:, qi:qi + 1])
                    else:
                        nc.vector.tensor_scalar_add(oslice, ps_o[:, :w],
                                                    bq_sb[:, qi:qi + 1])
                    # batch 1's stores go on the sync queue (its loads are
                    # done by then), decoupling DMA issue from the Act
                    # engine which is busy with psum evacuations
                    dma_eng = nc.sync if bi == 1 else nc.scalar
                    dma_eng.dma_start(
                        out=out_r[bi, :, qi, lo:lo + w],
                        in_=oslice)

            # ---- HAM warm-up on a memset tile (no DMA dependency).
            # ~6k cycles of dummy matmuls: the hw clock governor needs
            # sustained PE work before it ramps to 2.4 GHz (measured:
            # a 20-instruction warmup cost +8us of slow-clock matmuls).
            warm_sb = const.tile([P, P], BF16, name="warm_sb")
            nc.vector.memset(warm_sb, 0.0)
            ps_w = psO.tile([P, XCH], F32, tag="po", name="warm")
            NWARM = 44
            for wj in range(NWARM):
                nc.tensor.matmul(ps_w[:, :P], warm_sb, warm_sb,
                                 start=(wj == 0), stop=(wj == NWARM - 1))
            # warm the Exp activation table while the PE warms up
            dummy_e = smp.tile([P, 1], F32, tag="rs", name="dummy_e")
            nc.scalar.activation(out=dummy_e, in_=warm_sb[:, 0:1],
                                 func=mybir.ActivationFunctionType.Exp)

            # ---- DMA spine ----
            issue_xtb_dma(0)
            issue_late_consts()
            issue_xtb_dma(1)
            issue_xbf_dma(0)
            issue_xbf_dma(1)

            # ---- PE schedule ----
            emit_G_start(0)
            for j in range(NCHK):
                emit_G_chunk(0, j)
            emit_G_evac(0)
            emit_G_start(1)
            emit_G_chunk(1, 0)    # PE covers batch-0's row-evac latency
            emit_G_lower(0)
            emit_WG(0)
            emit_softmax(0)
            emit_G_chunk(1, 1)    # fill softmax(0) latency
            emit_G_chunk(1, 2)
            emit_ATcombine(0)
            emit_F(0, 0, 0)
            emit_G_chunk(1, 3)
            emit_G_chunk(1, 4)
            emit_F(0, 0, 1)
            emit_G_chunk(1, 5)
            emit_G_chunk(1, 6)
            emit_G_chunk(1, 7)
            emit_G_evac(1)
            emit_F(0, 1, 0)       # PE covers batch-1's row-evac latency
            emit_G_lower(1)
            emit_WG(1)
            emit_softmax(1)
            emit_F(0, 1, 1)       # covers softmax(1) latency
            emit_ATcombine(1)
            emit_F(1, 0, 0, deep=True)
            emit_F(1, 0, 1, deep=True)
            emit_F(1, 1, 0, deep=True)
            emit_F(1, 1, 1, deep=True, fine=True)
    nc.compile()
    return nc


_NC_CACHE = None


def _get_nc():
    global _NC_CACHE
    if _NC_CACHE is None:
        _NC_CACHE = build_nc()
    return _NC_CACHE


def make_in_maps(x, conv_w, conv_b, gamma):
    B = x.shape[0]
    xs = np.ascontiguousarray(x.reshape(B, C, HW), dtype=np.float32)
    # natural layout, p = c % 128 partition: [B, P, CT, HW]
    xn = xs.reshape(B, CT, P, HW).transpose(0, 2, 1, 3)
    xbf = np.ascontiguousarray(xn).astype(ml_dtypes.bfloat16)
    # transposed layout, p = n % 128 partition: [B, P, NT, C]
    xt = xs.transpose(0, 2, 1).reshape(B, NT, P, C).transpose(0, 2, 1, 3)
    xtb = np.ascontiguousarray(xt).astype(ml_dtypes.bfloat16)
    # per-batch channel row-sums for the rank-1 bias term of E
    ss = xs.sum(axis=2).astype(ml_dtypes.bfloat16)     # [B, C]

    wm = conv_w.reshape(C1, C).astype(np.float32)
    wt = np.ascontiguousarray(wm.T)                    # [C, C1]
    wt_tiled = np.ascontiguousarray(
        wt.reshape(CT, P, C1).transpose(1, 0, 2))      # [P, CT, C1]
    b_np = conv_b.astype(np.float32)
    bq = np.ascontiguousarray(b_np.reshape(QT, P).T)   # [P, QT]
    bb = np.ascontiguousarray(b_np[None, :]).astype(ml_dtypes.bfloat16)
    gam = np.ascontiguousarray(
        np.broadcast_to(gamma.astype(np.float32).reshape(1, 1), (P, 1)))
    in_maps = []
    for ci in range(N_CORES):
        sl = slice(NB * ci, NB * (ci + 1))
        in_maps.append({
            "xbfd": np.ascontiguousarray(xbf[sl]),
            "xtbd": np.ascontiguousarray(xtb[sl]),
            "wtfd": wt_tiled,
            "wthd": wt_tiled.astype(np.float16),
            "bqd": bq,
            "gamd": gam,
            "bbd": bb,
            "ssd": np.ascontiguousarray(ss[sl]).reshape(1, NB, C),
        })
    return in_maps


def kernel(x, conv_w, conv_b, gamma, trace=False):
    """Full inputs in, full output out. Shards batch over 8 NeuronCores."""
    nc = _get_nc()
    in_maps = make_in_maps(x, conv_w, conv_b, gamma)
    res = run_bass_kernel_spmd(nc, in_maps, core_ids=list(range(N_CORES)),
                               trace=trace)
    outs = [np.asarray(r["out"]).astype(np.float32).reshape(NB, C1, 64, 64)
            for r in res.results]
    full = np.concatenate(outs, axis=0)
    if trace:
        kernel.last_results = res
    return full


kernel.last_results = None
